# revision 22
# baseline (speedup 1.0000x reference)
"""Causal self-attention for trn2, 8 NeuronCores.

Problem: x[4,2048,1024] @ w_qkv[1024,3072] -> causal MHA (16 heads, d=64)
-> @ w_out[1024,1024].

Sharding: core c handles batch b=c%4 and heads hbase=8*(c//4)..hbase+8
(data parallel on B x tensor parallel on heads). Each core computes the
partial out-projection y_c = att_slice @ w_out[slice]; the host sums the
two partials per batch.

v8: inputs arrive host-cast to bf16 (numpy RTNE, same numerics as the
on-device casts they replace) and x arrives host-TRANSPOSED, so every
xT tile is a plain fast DMA slice (the on-device DMA-transpose path ran
at ~4.5us per [512,128] tile and paced the whole projection pipeline).
Attention processes a head-pair per kt tile: the two K=64 score matmuls
go to PE row-groups (0,0)/(64,0) back-to-back and execute concurrently
(row tiling), one exp covers both heads' [128,1024] scores, then both
AV matmuls follow. On diagonal blocks the fully-masked q-columns are
skipped end-to-end (sliced score-matmul N, 2D-AP exp, 128-wide causal
select band, sliced AV stream). Softmax denominators ride a fused
ones-column of V (row 64 of av); the reciprocal runs on a [64,8]
DRAM-bounced reshape so the 8-cyc/elem DVE op costs ~0.2us instead of
3.3us (which used to stall the DVE FIFO and starve the PE). Projection
work is woven into the ScalarE-paced attention at emission level: a
task FIFO holds this round's g1-3/V projection tiles, the next round's
g0 q/k tiles, and the previous round's out-projection tiles, and one
task is emitted after each attention burst so the in-order PE queue
alternates burst/tile; out-projection tasks drain last, after their
producer group's normalize chain has finished. The last group's
normalize chain rides the scalar DMA queue (idle at round tails).
Round-0's V tiles are all staged in the prologue: weaving them
just-in-time (1-burst margin) raced the AV weight loads on cold first
runs.
"""

import sys

for p in ("/opt/trn_rl_repo", "/opt/pypackages"):
    if p not in sys.path:
        sys.path.insert(0, p)

import contextlib
from collections import deque

import numpy as np

import concourse.bass as bass
import concourse.mybir as mybir
import concourse.tile as tile
from concourse import bacc
from concourse.bass_utils import run_bass_kernel_spmd

F32 = mybir.dt.float32
BF = mybir.dt.bfloat16
EXP = mybir.ActivationFunctionType.Exp

T = 2048          # sequence length
C = 1024          # model dim
HC = 8            # heads per core
D = 64            # head dim
NG = 4            # head-groups of 2 per core
NCT = C // 128    # 8 contraction tiles
NTT = T // 128    # 16 token tiles
SCALE = 0.125     # 1/sqrt(D)


def build_nc():
    nc = bacc.Bacc("TRN2", target_bir_lowering=False, debug=False)

    xT_d = nc.dram_tensor("xT", [C, T], BF, kind="ExternalInput")
    wq_d = nc.dram_tensor("wq", [C, 512], BF, kind="ExternalInput")
    wk_d = nc.dram_tensor("wk", [C, 512], BF, kind="ExternalInput")
    wv_d = nc.dram_tensor("wv", [C, 512], BF, kind="ExternalInput")
    wo_d = nc.dram_tensor("wo", [512, C], BF, kind="ExternalInput")
    y_d = nc.dram_tensor("y", [T, C], F32, kind="ExternalOutput")

    with tile.TileContext(nc) as tc, contextlib.ExitStack() as ctx:
        persist = ctx.enter_context(tc.tile_pool(name="persist", bufs=1))
        work = ctx.enter_context(tc.tile_pool(name="work", bufs=1))
        ps = ctx.enter_context(tc.tile_pool(name="ps", bufs=1, space="PSUM"))
        dpool = ctx.enter_context(tc.tile_pool(name="dram", bufs=1, space="DRAM"))

        kT = [persist.tile([128, T], BF, tag=f"kT{g}", name=f"kT{g}")
              for g in range(NG)]
        V = persist.tile([128, NTT, HC, 65], BF, tag="V")

        # weights: per-ct bf16 loads on the scalar HWDGE queue (parallel to
        # the x transposes on sync) so the first projection matmul can start
        # as soon as wq[ct=0] lands.
        wq_bf = persist.tile([128, NCT, 512], BF, tag="wq_bf")
        wk_bf = persist.tile([128, NCT, 512], BF, tag="wk_bf")
        wv_bf = persist.tile([128, NCT, 512], BF, tag="wv_bf")
        for wdram, wbf in ((wq_d, wq_bf), (wk_d, wk_bf), (wv_d, wv_bf)):
            for ct in range(NCT):
                nc.scalar.dma_start(
                    out=wbf[:, ct, :],
                    in_=wdram.ap()[ct * 128:(ct + 1) * 128, :])
        wo_bf = persist.tile([128, NG, C], BF, tag="wo_bf")
        nc.scalar.dma_start(
            out=wo_bf, in_=wo_d.ap().rearrange("(g p) c -> p g c", p=128))

        # ones column of V
        ones_f32 = persist.tile([128, NTT, HC], F32, tag="ones")
        nc.vector.memset(ones_f32, 1.0)
        nc.vector.tensor_copy(V[:, :, :, 64], ones_f32)

        def issue_xt_loads(rnd):
            q0 = rnd * 512
            xTq = [work.tile([128, 512], BF, tag=f"xTq{ct}",
                             name=f"xTq{ct}", bufs=2)
                   for ct in range(NCT)]
            for ct in range(NCT):
                nc.sync.dma_start(
                    out=xTq[ct],
                    in_=xT_d.ap()[ct * 128:(ct + 1) * 128, q0:q0 + 512]
                )
            return xTq

        qTq_by_round = {r: [None] * NG for r in range(4)}

        def make_proj_tasks(rnd, xTq):
            q0 = rnd * 512

            def tq(g):
                def run():
                    pq = ps.tile([128, 512], F32, tag="pp", bufs=2, name="pq")
                    for ct in range(NCT):
                        nc.tensor.matmul(
                            pq,
                            wq_bf[:, ct, g * 128:(g + 1) * 128],
                            xTq[ct],
                            start=(ct == 0), stop=(ct == NCT - 1),
                        )
                    qq = work.tile([128, 512], BF, tag=f"qTq{g}", bufs=2,
                                   name=f"qTq{g}")
                    nc.vector.tensor_copy(qq, pq)
                    qTq_by_round[rnd][g] = qq
                return run

            def tk(g):
                def run():
                    pk = ps.tile([128, 512], F32, tag="pp", bufs=2, name="pk")
                    for ct in range(NCT):
                        nc.tensor.matmul(
                            pk,
                            wk_bf[:, ct, g * 128:(g + 1) * 128],
                            xTq[ct],
                            start=(ct == 0), stop=(ct == NCT - 1),
                        )
                    nc.vector.tensor_copy(kT[g][:, q0:q0 + 512], pk)
                return run

            def tv(sub):
                def run():
                    pv = ps.tile([128, 512], F32, tag="pp", bufs=2, name="pv")
                    for ct in range(NCT):
                        nc.tensor.matmul(
                            pv,
                            xTq[ct][:, sub * 128:(sub + 1) * 128],
                            wv_bf[:, ct, :],
                            start=(ct == 0), stop=(ct == NCT - 1),
                        )
                    tt = rnd * 4 + sub
                    nc.vector.tensor_copy(
                        V[:, tt, :, 0:64],
                        pv[:, :].rearrange("p (h d) -> p h d", d=64),
                    )
                return run

            return tq, tk, tv

        def make_out_tasks(rnd, att_tiles):
            def t(qtl):
                def run():
                    qt = rnd * 4 + qtl
                    y_sb = work.tile([128, C], F32, tag="y_sb", bufs=2,
                                     name="y_sb")
                    for half in range(2):
                        psy = ps.tile([128, 512], F32, tag="pp", bufs=2,
                                      name="psy")
                        for g in range(NG):
                            nc.tensor.matmul(
                                psy,
                                att_tiles[g][:, qtl * 128:(qtl + 1) * 128],
                                wo_bf[:, g, half * 512:(half + 1) * 512],
                                start=(g == 0),
                                stop=(g == NG - 1),
                            )
                        nc.vector.tensor_copy(
                            y_sb[:, half * 512:(half + 1) * 512], psy)
                    nc.sync.dma_start(
                        out=y_d.ap()[qt * 128:(qt + 1) * 128, :], in_=y_sb
                    )
                return run
            return [t(qtl) for qtl in range(4)]

        pending = deque()
        attTq_prev = None
        xTq_cur = issue_xt_loads(0)
        tq0, tk0, tv0 = make_proj_tasks(0, xTq_cur)
        # prologue: g0's q/k and quarter 0's V (needed from burst 0 on)
        tq0(0)(); tk0(0)()
        for s in range(4):
            tv0(s)()
        for rnd in range(4):
            # own deferred projections (g1-3 q/k, and V for rnd>0) drain in
            # this round's early bursts; next round's g0 q/k follows; the
            # previous round's out-projection drains last, well after its
            # g3 normalize chain has finished.
            if rnd == 0:
                tq_c, tk_c, tv_c = tq0, tk0, tv0
                pending.extend([tq_c(1), tk_c(1), tq_c(2), tk_c(2),
                                tq_c(3), tk_c(3)])
            elif rnd == 1:
                tq_c, tk_c, tv_c = make_proj_tasks(rnd, xTq_cur)
                pending.extend([tq_c(1), tk_c(1), tq_c(2), tk_c(2),
                                tq_c(3), tk_c(3)])
            else:
                tq_c, tk_c, tv_c = make_proj_tasks(rnd, xTq_cur)
                pending.extend([tq_c(1), tk_c(1), tv_c(0), tv_c(1),
                                tv_c(2), tv_c(3), tq_c(2), tk_c(2),
                                tq_c(3), tk_c(3)])
            if rnd < 3:
                xTq_next = issue_xt_loads(rnd + 1)
                tq_n, tk_n, tv_n = make_proj_tasks(rnd + 1, xTq_next)
                pending.extend([tq_n(0), tk_n(0)])
                if rnd == 0:
                    pending.extend([tv_n(s) for s in range(4)])
                xTq_cur = xTq_next
            if attTq_prev is not None:
                pending.extend(make_out_tasks(rnd - 1, attTq_prev))

            # ---- attention: q-block rnd for every head-pair ----
            qTq = qTq_by_round[rnd]
            nkt = 4 * (rnd + 1)
            attTq = []
            for g in range(NG):
                att = work.tile([128, 512], BF, tag=f"attTq{g}", bufs=2,
                                name=f"attTq{g}")
                av0 = ps.tile([65, 512], F32, tag="av0", name="av0")
                av1 = ps.tile([65, 512], F32, tag="av1", name="av1")
                for kt in range(nkt):
                    j = kt - 4 * rnd  # >=0 on diagonal 128-blocks
                    c0 = 128 * j if j > 0 else 0  # fully-masked q-columns
                    sc = ps.tile([128, 1024], F32, tag="sc", bufs=2, name="sc")
                    nc.tensor.matmul(
                        sc[:, c0:512],
                        kT[g][0:64, kt * 128:(kt + 1) * 128],
                        qTq[g][0:64, c0:512],
                        start=True, stop=True,
                        tile_position=(0, 0),
                    )
                    nc.tensor.matmul(
                        sc[:, 512 + c0:1024],
                        kT[g][64:128, kt * 128:(kt + 1) * 128],
                        qTq[g][64:128, c0:512],
                        start=True, stop=True,
                        tile_position=(64, 0),
                    )
                    wT = work.tile([128, 1024], BF, tag="wT", bufs=3)
                    if c0:
                        nc.scalar.activation(
                            wT[:, :].rearrange("p (m c) -> p m c", m=2)
                                    [:, :, c0:512],
                            sc[:, :].rearrange("p (m c) -> p m c", m=2)
                                    [:, :, c0:512],
                            EXP, scale=SCALE)
                    else:
                        nc.scalar.activation(wT, sc, EXP, scale=SCALE)
                    if j >= 0:  # causal select on the 128-wide boundary band
                        for m in range(2):
                            b0 = m * 512 + c0
                            nc.gpsimd.affine_select(
                                out=wT[:, b0:b0 + 128],
                                in_=wT[:, b0:b0 + 128],
                                compare_op=mybir.AluOpType.is_ge,
                                fill=0.0,
                                base=0,
                                pattern=[[1, 128]],
                                channel_multiplier=-1,
                            )
                    nc.tensor.matmul(
                        av0[:, c0:512], V[:, kt, 2 * g, :], wT[:, c0:512],
                        start=(kt == 0), stop=(kt == nkt - 1),
                    )
                    nc.tensor.matmul(
                        av1[:, c0:512], V[:, kt, 2 * g + 1, :],
                        wT[:, 512 + c0:1024],
                        start=(kt == 0), stop=(kt == nkt - 1),
                    )
                    if pending:
                        pending.popleft()()
                # normalization, staged off PSUM so the accumulators free up.
                # The last group's chain rides the scalar DMA queue: ScalarE
                # is idle at round tails and the sync queue is congested.
                dqs = ((nc.scalar, nc.sync) if g == NG - 1
                       else (nc.sync, nc.sync))
                for hh, av in ((0, av0), (1, av1)):
                    dq = dqs[hh]
                    avc = work.tile([65, 512], F32, tag="avc", bufs=4,
                                    name="avc")
                    nc.vector.tensor_copy(avc, av)
                    # reciprocal on a [64,8] DRAM-bounced reshape: the
                    # 8-cyc/elem DVE reciprocal runs on 8 columns x 64 lanes
                    den_d = dpool.tile([1, 512], F32, tag="den_d", bufs=32,
                                       name="den_d")
                    dq.dma_start(out=den_d, in_=avc[64:65, :])
                    d8 = work.tile([64, 8], F32, tag="d8", bufs=32, name="d8")
                    dq.dma_start(
                        out=d8,
                        in_=bass.AP(den_d.tensor, den_d.offset,
                                    [[8, 64], [1, 8]]),
                    )
                    r8 = work.tile([64, 8], F32, tag="r8", bufs=32, name="r8")
                    nc.vector.reciprocal(r8, d8)
                    rec_d = dpool.tile([1, 512], F32, tag="rec_d", bufs=32,
                                       name="rec_d")
                    dq.dma_start(
                        out=bass.AP(rec_d.tensor, rec_d.offset,
                                    [[8, 64], [1, 8]]),
                        in_=r8,
                    )
                    rep = work.tile([64, 512], F32, tag="rep", bufs=4,
                                    name="rep")
                    dq.dma_start(
                        out=rep,
                        in_=bass.AP(rec_d.tensor, rec_d.offset,
                                    [[0, 64], [1, 512]]),
                    )
                    if hh == 0:
                        nc.vector.tensor_mul(att[0:64, :], avc[0:64, :], rep)
                    else:
                        tmpB = work.tile([64, 512], BF, tag="tmpB", bufs=2,
                                         name="tmpB")
                        nc.vector.tensor_mul(tmpB, avc[0:64, :], rep)
                        dq.dma_start(out=att[64:128, :], in_=tmpB)
                attTq.append(att)
            if rnd == 3:
                for t in make_out_tasks(3, attTq):
                    t()
            while pending:
                pending.popleft()()
            attTq_prev = attTq

    nc.compile()
    return nc


_NC_CACHE = None


def _get_nc():
    global _NC_CACHE
    if _NC_CACHE is None:
        _NC_CACHE = build_nc()
    return _NC_CACHE


def kernel(x, w_qkv, w_out, _trace=False):
    import ml_dtypes

    B = x.shape[0]
    bf16 = ml_dtypes.bfloat16
    x = np.asarray(x, dtype=np.float32).astype(bf16)
    w_qkv = np.asarray(w_qkv, dtype=np.float32).astype(bf16)
    w_out = np.asarray(w_out, dtype=np.float32).astype(bf16)

    nc = _get_nc()
    in_maps = []
    for core in range(8):
        b = core % B
        hbase = (core // B) * HC
        lo, hi = hbase * D, hbase * D + HC * D
        in_maps.append({
            "xT": np.ascontiguousarray(x[b].T),
            "wq": np.ascontiguousarray(w_qkv[:, lo:hi]),
            "wk": np.ascontiguousarray(w_qkv[:, C + lo:C + hi]),
            "wv": np.ascontiguousarray(w_qkv[:, 2 * C + lo:2 * C + hi]),
            "wo": np.ascontiguousarray(w_out[lo:hi, :]),
        })

    res = run_bass_kernel_spmd(nc, in_maps, core_ids=list(range(8)), trace=_trace)
    ys = [r["y"] for r in res.results]
    out = np.empty((B, T, C), dtype=np.float32)
    for b in range(B):
        out[b] = ys[b] + ys[b + B]
    if _trace:
        return out, res
    return out
